# revision 94
# baseline (speedup 1.0000x reference)
"""GQA attention kernel for Trainium2, 8 NeuronCores.

Sharding: data-parallel over batch (B=2) x tensor-parallel over KV heads
(HKV=4) -> 8 cores.  Core c handles batch b=c//4, kv-head j=c%4 with its
G=4 query heads.  out_proj is row-parallel; partials are reduced on host.

Layout strategy (v2):
  - Projections in NATURAL orientation (out[seq, feat]): lhsT = hiddenT
    chunk, rhs = W chunk.  RoPE and rmsnorm then operate along the free
    dim (cheap DVE/Pool ops, no partition reductions).
  - rsqrt for rmsnorm is exp(-0.5*ln(x)) on ACT: both funcs live in the
    natural_log_exp_and_others activation table together with the softmax
    Exp, so the ACT engine never reloads its table.
  - qT / kT for the scores matmuls are produced by DMA-transpose
    (crossbar) instructions; kT's row-64..127 duplicate is folded into
    the same transpose by duplicating kn columns beforehand.
  - scoresT[key, q] = kT^T @ qT per head, exp on ACT (the hard floor:
    ~110us of exp at 0.833 ns/elem), probabilities pT kept in SBUF for a
    full 512-q block.
  - PV in flipped orientation: out[q, d+1] with lhsT = pT chunk,
    rhs = v (with ones column -> denominator lands as column 64).  N=65
    per matmul instead of 512 -> half the PE rows of the baseline.
    Normalization is a per-partition reciprocal + broadcast multiply.
  - oT via DMA-transpose feeds a row-parallel out_proj; partials DMA'd
    per 128-row chunk.
PSUM budget (8 banks): pq 1 | scoresA 2 | scoresB 2 | oraw 2 | y 1.
The lead-in k/v/q chains round-robin across all five slots.
"""

import numpy as np
import ml_dtypes

import concourse.bacc as bacc
import concourse.mybir as mybir
from concourse.tile import TileContext

BF16 = mybir.dt.bfloat16
F32 = mybir.dt.float32
U32 = mybir.dt.uint32
AL = mybir.AluOpType
AF = mybir.ActivationFunctionType
AX = mybir.AxisListType

B, S, HID = 2, 2048, 1024
H, HKV, D = 16, 4, 64
G = H // HKV          # 4 query heads per kv head
QSEL = 2 * G * D      # 512: own 256 cols + rope-partner 256 cols
ROPE_BASE = 10000.0
EPS = float(np.finfo(np.float32).eps)
NSC = S // 128        # 16 seq chunks
NIC = 4               # 512-wide q blocks

NB = ml_dtypes.bfloat16

_cache: dict = {}


def _build(use_mask: bool, use_bias: bool, debug: bool = False):
    nc = bacc.Bacc("TRN2", target_bir_lowering=False)

    hT = nc.dram_tensor("hT", [8, 128, S], BF16, kind="ExternalInput")
    wq = nc.dram_tensor("wq", [8, 128, QSEL], BF16, kind="ExternalInput")
    wkv = nc.dram_tensor("wkv", [8, 128, 192], BF16, kind="ExternalInput")
    wo = nc.dram_tensor("wo", [2, 128, HID], BF16, kind="ExternalInput")
    csq = nc.dram_tensor("csq", [NSC, 128, 2, 256], BF16, kind="ExternalInput")
    csk = nc.dram_tensor("csk", [NSC, 128, 2, 64], BF16, kind="ExternalInput")
    y = nc.dram_tensor("y", [NSC, 128, HID], BF16, kind="ExternalOutput")
    mk = (
        nc.dram_tensor("mk", [NSC, 128, S], F32, kind="ExternalInput")
        if use_mask
        else None
    )
    if use_bias:
        brq = nc.dram_tensor("brq", [1, QSEL], BF16, kind="ExternalInput")
        brkv = nc.dram_tensor("brkv", [1, 192], BF16, kind="ExternalInput")
    if debug:
        d_qT = nc.dram_tensor("d_qT", [128, 2, S], BF16, kind="ExternalOutput")
        d_kT = nc.dram_tensor("d_kT", [128, S], BF16, kind="ExternalOutput")
        d_rk = nc.dram_tensor("d_rk", [128, NSC], F32, kind="ExternalOutput")
        d_v = nc.dram_tensor("d_v", [128, NSC, 66], BF16, kind="ExternalOutput")
        d_pT = nc.dram_tensor("d_pT", [3, 128, 4, 512], BF16, kind="ExternalOutput")
        d_o = nc.dram_tensor("d_o", [128, 2, 4, 64], BF16, kind="ExternalOutput")

    with TileContext(nc) as tc:
        with (
            tc.tile_pool(name="const", bufs=1) as cp,
            tc.tile_pool(name="proj", bufs=1) as pj,
            tc.tile_pool(name="rt", bufs=3) as rt,
            tc.tile_pool(name="ro", bufs=6) as rop,
            tc.tile_pool(name="stat", bufs=3) as stp,
            tc.tile_pool(name="pT", bufs=25) as ptp,
            tc.tile_pool(name="onat", bufs=3) as onp_,
            tc.tile_pool(name="oTp", bufs=3) as otp,
            tc.tile_pool(name="ysb", bufs=3) as yp,
            tc.tile_pool(name="maskp", bufs=3) as mp,
            tc.tile_pool(name="ps", bufs=1, space="PSUM") as ps,
        ):
            # ---- persistent tiles ------------------------------------
            wo_sb = cp.tile([128, 2, HID], BF16)
            for cc in range(2):
                nc.sync.dma_start(out=wo_sb[:, cc, :], in_=wo[cc])
            v_sb = cp.tile([128, NSC, 66], BF16)
            nc.vector.memset(v_sb[:, :, 64:65], 1.0)
            qT = cp.tile([128, 2, S], BF16)
            kT = cp.tile([128, S], BF16)
            # per-key 1/(8*rms(k)) factors, applied as the exp scale
            rkinv8 = cp.tile([128, NSC], F32)

            # ---- projection-phase constants --------------------------
            # hidden-state chunks first: they gate the tail of every
            # projection's accumulation; the small weights hide behind them
            hT_sb = pj.tile([128, 8, S], BF16)
            nc.sync.dma_start(out=hT_sb[:, 0, :], in_=hT[0])
            wkv_sb = pj.tile([128, 8, 192], BF16)
            nc.sync.dma_start(out=wkv_sb[:], in_=wkv[:].rearrange("a b c -> b a c"))
            for ko in range(1, 8):
                nc.sync.dma_start(out=hT_sb[:, ko, :], in_=hT[ko])
            wq_sb = pj.tile([128, 8, QSEL], BF16)
            nc.sync.dma_start(out=wq_sb[:], in_=wq[:].rearrange("a b c -> b a c"))
            csk_sb = pj.tile([128, NSC, 2, 64], BF16)
            nc.sync.dma_start(out=csk_sb[:], in_=csk[:].rearrange("a b c d -> b a c d"))
            csq_sb = pj.tile([128, NSC, 2, 256], BF16)
            nc.sync.dma_start(
                out=csq_sb[:, 0:4, :, :],
                in_=csq[0:4].rearrange("a b c d -> b a c d"),
            )
            nc.sync.dma_start(
                out=csq_sb[:, 4:NSC, :, :],
                in_=csq[4:NSC].rearrange("a b c d -> b a c d"),
            )
            if use_bias:
                ones1 = cp.tile([1, 128], BF16)
                nc.vector.memset(ones1[:], 1.0)
                brq_sb = cp.tile([1, QSEL], BF16)
                nc.sync.dma_start(out=brq_sb[:], in_=brq[:])
                brkv_sb = cp.tile([1, 192], BF16)
                nc.sync.dma_start(out=brkv_sb[:], in_=brkv[:])

            # PSUM slot round-robin for the lead-in projection chains
            # ("py" is reserved for the dripped k/v groups 1..3)
            SLOTS = ["scA", "scB", "orA", "orB", "pq"]
            slot_i = [0]

            def next_slot():
                s = SLOTS[slot_i[0] % len(SLOTS)]
                slot_i[0] += 1
                return s

            def rsqrt_batch(rm, tag, eps=EPS, out=None, premul=None):
                """rm: [128, n] f32 -> (premul*rm+eps)^-0.5 on DVE (quake
                seed + 2 Newton steps, ~5e-6 rel err) so the ACT engine only
                ever runs Exp: a single table load per kernel."""
                e = nc.vector
                n = rm.shape[1]
                me = stp.tile([128, n], F32, tag=tag + "_me")
                if premul is None:
                    e.tensor_scalar_add(me[:], rm[:], eps)
                else:
                    e.tensor_scalar(me[:], rm[:], premul, eps, AL.mult, AL.add)
                u1 = stp.tile([128, n], U32, tag=tag + "_u1")
                e.tensor_scalar(u1[:], me[:].bitcast(U32), 1, None,
                                AL.arith_shift_right)
                g = stp.tile([128, n], U32, tag=tag + "_g")
                e.tensor_scalar(g[:], u1[:], 0x5F3759DF, -1,
                                AL.subtract, AL.mult)
                t1 = stp.tile([128, n], F32, tag=tag + "_t1")
                t2 = stp.tile([128, n], F32, tag=tag + "_t2")
                ya = stp.tile([128, n], F32, tag=tag + "_ya")
                rc = out if out is not None else stp.tile([128, n], F32,
                                                          tag=tag + "_rc", name="rc")
                cur = g[:].bitcast(F32)
                for ynext in (ya, rc):
                    e.tensor_tensor(t1[:], cur, cur, AL.mult)
                    e.scalar_tensor_tensor(t2[:], me[:], -0.5, t1[:],
                                           AL.mult, AL.mult)
                    e.scalar_tensor_tensor(ynext[:], t2[:], 1.5, cur,
                                           AL.add, AL.mult)
                    cur = ynext[:]
                return rc

            def kv_mm(sc, box, ko0, slot):
                """four accumulation matmuls of the merged k|v projection"""
                ssl = slice(sc * 128, (sc + 1) * 128)
                if ko0 == 0:
                    box.append(ps.tile([128, 192], F32, tag=slot, name="pkv"))
                pkv = box[0]
                for ko in range(ko0, ko0 + 4):
                    st, sp = ko == 0, (ko == 7 and not use_bias)
                    nc.tensor.matmul(
                        pkv[:], lhsT=hT_sb[:, ko, ssl], rhs=wkv_sb[:, ko, :],
                        start=st, stop=sp,
                    )
                if ko0 == 4 and use_bias:
                    nc.tensor.matmul(pkv[:], lhsT=ones1[:], rhs=brkv_sb[:],
                                     start=False, stop=True)

            def kv_tail(sc, rmk4, i, pkv):
                """rope k, transpose it UNNORMALIZED (the 1/(8 rms) factor is
                applied later as the exp instruction's per-partition scale),
                accumulate sum(k^2), stash v."""
                t12k = rt.tile([128, 2, 64], BF16, tag="t12k")
                nc.vector.tensor_tensor(
                    t12k[:], pkv[:, 0:128].rearrange("p (c d) -> p c d", c=2),
                    csk_sb[:, sc, :, :], AL.mult,
                )
                kro = rop.tile([128, 64], BF16, tag="kro")
                nc.gpsimd.tensor_tensor(kro[:], t12k[:, 0, :], t12k[:, 1, :], AL.add)
                kn2 = rt.tile([128, 2, 64], BF16, tag="kn2")
                nc.vector.tensor_copy(
                    kn2[:], kro[:, None, :].to_broadcast((128, 2, 64))
                )
                nc.sync.dma_start_transpose(
                    out=kT[:, sc * 128:(sc + 1) * 128], in_=kn2[:]
                )
                sqk = rt.tile([128, 64], BF16, tag="sqk")
                nc.gpsimd.tensor_tensor(sqk[:], kro[:], kro[:], AL.mult)
                nc.vector.tensor_reduce(rmk4[:, i:i + 1], sqk[:], AX.X, AL.add)
                nc.vector.tensor_copy(v_sb[:, sc, 0:64], pkv[:, 128:192])

            def kv_sub(sc, rmk4, i):
                box = []
                kv_mm(sc, box, 0, next_slot())
                kv_mm(sc, box, 4, None)
                kv_tail(sc, rmk4, i, box[0])

            def kv_fin(g, rmk4):
                # sum(k^2) + 64 eps -> rsqrt gives 1/(8 rms(k))
                rsqrt_batch(rmk4, "rck", eps=64.0 * EPS,
                            out=rkinv8[:, 4 * g:4 * g + 4])

            def q_sub(sc, rms16, i, qros, lead=False):
                ssl = slice(sc * 128, (sc + 1) * 128)
                pq = ps.tile([128, 2, 256], F32, tag=(next_slot() if lead else "pq"))
                for ko in range(8):
                    st, sp = ko == 0, (ko == 7 and not use_bias)
                    nc.tensor.matmul(
                        pq[:], lhsT=hT_sb[:, ko, ssl], rhs=wq_sb[:, ko, :],
                        start=st, stop=sp,
                    )
                if use_bias:
                    nc.tensor.matmul(pq[:], lhsT=ones1[:], rhs=brq_sb[:],
                                     start=False, stop=True)
                q_sub_tail(sc, rms16, i, qros, pq)

            def q_sub_mm(sc, pq_box, ko0, lead=False):
                """two accumulation matmuls of the q projection for chunk sc"""
                ssl = slice(sc * 128, (sc + 1) * 128)
                if ko0 == 0:
                    pq_box.append(
                        ps.tile([128, 2, 256], F32,
                                tag=(next_slot() if lead else "pq"), name="pq")
                    )
                pq = pq_box[0]
                for ko in (ko0, ko0 + 1):
                    st = ko == 0
                    sp = ko == 7 and not use_bias
                    nc.tensor.matmul(
                        pq[:], lhsT=hT_sb[:, ko, ssl], rhs=wq_sb[:, ko, :],
                        start=st, stop=sp,
                    )
                if ko0 == 6 and use_bias:
                    nc.tensor.matmul(pq[:], lhsT=ones1[:], rhs=brq_sb[:],
                                     start=False, stop=True)

            def q_sub_tail(sc, rms16, i, qros, pq):
                t12 = rt.tile([128, 2, 256], BF16, tag="t12")
                nc.vector.tensor_tensor(t12[:], pq[:], csq_sb[:, sc, :, :], AL.mult)
                qro = rop.tile([128, 4, 64], BF16, tag="qro")
                nc.gpsimd.tensor_tensor(
                    qro[:].rearrange("p h d -> p (h d)"), t12[:, 0, :], t12[:, 1, :],
                    AL.add,
                )
                qros.append(qro)
                sqq = rt.tile([128, 4, 64], BF16, tag="sqq")
                nc.gpsimd.tensor_tensor(sqq[:], qro[:], qro[:], AL.mult)
                nc.vector.tensor_reduce(rms16[:, 4 * i:4 * i + 4], sqq[:], AX.X, AL.add)

            def q_fin(ic, rms16, qros):
                rcq = rsqrt_batch(rms16, "rcq", premul=1.0 / 64.0)
                for i in range(4):
                    sc = 4 * ic + i
                    qn = rt.tile([128, 4, 64], BF16, tag="qn")
                    nc.vector.tensor_tensor(
                        qn[:], qros[i][:],
                        rcq[:, 4 * i:4 * i + 4, None].to_broadcast((128, 4, 64)),
                        AL.mult,
                    )
                    nc.sync.dma_start_transpose(
                        out=qT[:, :, sc * 128:(sc + 1) * 128], in_=qn[:]
                    )

            def q_fin1(sc, rms16, i, qro):
                """per-chunk finalize for the lead-in: no cross-chunk sync,
                transpose issued on the (idle) ACT hwdge queue."""
                rcq = rsqrt_batch(rms16[:, 4 * i:4 * i + 4], "rcq" + str(i % 2),
                                  premul=1.0 / 64.0)
                qn = rt.tile([128, 4, 64], BF16, tag="qn")
                nc.vector.tensor_tensor(
                    qn[:], qro[:],
                    rcq[:, :, None].to_broadcast((128, 4, 64)), AL.mult,
                )
                nc.scalar.dma_start_transpose(
                    out=qT[:, :, sc * 128:(sc + 1) * 128], in_=qn[:]
                )

            def pv_one(pts, sub, hd, bank_tag, onat_box, dbg_ic0):
                """one PV accumulation group: exclusive psum bank, 16
                back-to-back matmuls over the cached pT tiles, then
                normalize its head slice of the sub's o tile."""
                def run():
                    orw = ps.tile([128, 128], F32, tag=bank_tag, name="orw")
                    for jc in range(16):
                        nc.tensor.matmul(
                            orw[:, 0:65],
                            lhsT=pts[jc][:, hd, sub * 128:(sub + 1) * 128],
                            rhs=v_sb[:, jc, 0:65],
                            start=(jc == 0), stop=(jc == 15),
                        )
                    rcp = stp.tile([128, 1], F32, tag="rcp", name="rcp")
                    nc.vector.reciprocal(rcp[:], orw[:, 64:65])
                    if not onat_box:
                        onat_box.append(
                            onp_.tile([128, 4, 64], BF16, tag="onat", name="onat")
                        )
                    nc.vector.tensor_scalar_mul(
                        onat_box[0][:, hd, :], orw[:, 0:64], rcp[:]
                    )
                    if debug and dbg_ic0 and sub < 2:
                        nc.sync.dma_start(out=d_o[:, sub, hd, :],
                                          in_=onat_box[0][:, hd, :])
                return run

            def sub_fin(gc, onat_box, tail=False):
                def run():
                    oTt = otp.tile([128, 2, 128], BF16, tag="oTt", name="oTt")
                    eng = nc.scalar if tail else nc.sync
                    eng.dma_start_transpose(out=oTt[:], in_=onat_box[0][:])
                    outproj_sub(gc, oTt, tail=tail)
                return run

            def outproj_sub(gc, oTt, tail=False):
                ysb = yp.tile([128, HID], BF16, tag="ysb")
                for ec in range(2):
                    # after the last exp the scores slots and ACT engine are
                    # free: use them so the tail out_proj fully pipelines
                    py = ps.tile([128, 512], F32, name="py",
                                 tag=(("scA", "scB")[ec] if tail else "py"))
                    for cc in range(2):
                        nc.tensor.matmul(
                            py[:], lhsT=oTt[:, cc, :],
                            rhs=wo_sb[:, cc, ec * 512:(ec + 1) * 512],
                            start=(cc == 0), stop=(cc == 1),
                        )
                    if tail and ec == 0:
                        nc.scalar.copy(ysb[:, 0:512], py[:])
                    else:
                        nc.vector.tensor_copy(ysb[:, ec * 512:(ec + 1) * 512], py[:])
                nc.sync.dma_start(out=y[gc], in_=ysb[:])

            # ---- lead-in, ko-major: the projection matmuls for a wave of
            # chunks are emitted per contraction step so the PE stream tracks
            # the hidden-state chunk DMAs instead of serializing per chunk
            def proj_mm1(kind, sc, box, ko, slot):
                ssl = slice(sc * 128, (sc + 1) * 128)
                w, n = (wkv_sb, 192) if kind == "k" else (wq_sb, QSEL)
                if ko == 0:
                    box.append(ps.tile([128, n], F32, tag=slot, name="pj1"))
                nc.tensor.matmul(
                    box[0][:], lhsT=hT_sb[:, ko, ssl], rhs=w[:, ko, :],
                    start=(ko == 0), stop=(ko == 7 and not use_bias),
                )
                if ko == 7 and use_bias:
                    br = brkv_sb if kind == "k" else brq_sb
                    nc.tensor.matmul(box[0][:], lhsT=ones1[:], rhs=br[:],
                                     start=False, stop=True)

            rmk4 = stp.tile([128, 4], F32, tag="rmk4")
            rms16 = stp.tile([128, 16], F32, tag="rms16")
            qros = []
            kv_sub(0, rmk4, 0)
            kv_sub(1, rmk4, 1)
            q_sub(0, rms16, 0, qros, lead=True)
            kv_sub(2, rmk4, 2)
            q_sub(1, rms16, 1, qros, lead=True)
            kv_sub(3, rmk4, 3)
            q_sub(2, rms16, 2, qros, lead=True)
            kv_fin(0, rmk4)
            q_sub(3, rms16, 3, qros, lead=True)
            # one batched rsqrt: only the LAST chunk's chain is critical, and
            # batching keeps three extra Newton chains out of the in-order
            # DVE stream ahead of it
            rcq0 = rsqrt_batch(rms16, "rcq0", premul=1.0 / 64.0)
            for i in range(4):
                qn = rt.tile([128, 4, 64], BF16, tag="qn", name="qn")
                nc.vector.tensor_tensor(
                    qn[:], qros[i][:],
                    rcq0[:, 4 * i:4 * i + 4, None].to_broadcast((128, 4, 64)),
                    AL.mult,
                )
                nc.scalar.dma_start_transpose(
                    out=qT[:, :, i * 128:(i + 1) * 128], in_=qn[:]
                )

            # k/v groups 1..3 drip through the reserved "py" psum slot
            # during block 0 of the attention sweep
            from collections import deque
            wkv_ = deque()
            for g in range(1, 4):
                grmk = stp.tile([128, 4], F32, tag="rmk4", name="rmk4")
                for i in range(4):
                    sc = 4 * g + i
                    box = []
                    wkv_.append(
                        (lambda s, b: lambda: kv_mm(s, b, 0, "py"))(sc, box)
                    )
                    wkv_.append(
                        (lambda s, b: lambda: kv_mm(s, b, 4, None))(sc, box)
                    )
                    wkv_.append(
                        (lambda s, r, i2, b: lambda: kv_tail(s, r, i2, b[0]))(
                            sc, grmk, i, box
                        )
                    )
                wkv_.append((lambda g2, r: lambda: kv_fin(g2, r))(g, grmk))

            # ---- attention: exp spine + drip-fed side work -----------
            # Two FIFO queues keep non-spine work out of the scores->exp
            # chain's way: wpv (PV sweeps + normalize/out_proj, ordered to
            # respect the single oraw psum slot) and wq (next block's q
            # projection chain).
            from collections import deque
            wpv, wq_ = deque(), deque()

            def pump(q, n):
                for _ in range(n):
                    if not q:
                        return
                    q.popleft()()

            state = {}  # per-ic boxes

            for ic in range(NIC):
                isl = slice(ic * 512, (ic + 1) * 512)
                pts = []
                state[ic] = dict(pts=pts)
                # previous block's PV sweeps (one exclusive psum bank per
                # accumulation group) + per-sub finishes
                if ic > 0:
                    pv = state[ic - 1]
                    for sub in range(4):
                        ob = []
                        for hd in range(4):
                            wpv.append(pv_one(pv["pts"], sub, hd,
                                              ("orA", "orB")[(4 * sub + hd) % 2],
                                              ob, ic - 1 == 0))
                        wpv.append(sub_fin((ic - 1) * 4 + sub, ob))
                # next block's q projection chain
                if ic < 3:
                    nrms = stp.tile([128, 16], F32, tag="rms16")
                    nqros = []
                    for i in range(4):
                        sc = 4 * (ic + 1) + i
                        pq_box = []
                        for ko0 in (0, 2, 4, 6):
                            wq_.append(
                                (lambda s, b, k: lambda: q_sub_mm(s, b, k))(sc, pq_box, ko0)
                            )
                        wq_.append(
                            (lambda s, b, i2: lambda: q_sub_tail(s, nrms, i2, nqros, b[0]))(sc, pq_box, i)
                        )

                for jc in range(16):
                    pT_t = ptp.tile([128, 4, 512], BF16, tag="pT")
                    pts.append(pT_t)
                    if use_mask:
                        mkt = mp.tile([128, 512], F32, tag="mkt")
                        nc.sync.dma_start(out=mkt[:], in_=mk[jc][:, isl])
                    for pair in range(2):
                        pss = ps.tile([128, 2, 512], F32,
                                      tag=("scA" if pair == 0 else "scB"))
                        for hh in range(2):
                            rows = slice(64 * hh, 64 * hh + 64)
                            nc.tensor.matmul(
                                pss[:, hh, :],
                                lhsT=kT[rows, jc * 128:(jc + 1) * 128],
                                rhs=qT[rows, pair, isl],
                                start=True, stop=True,
                            )
                        if use_mask:
                            sm = mp.tile([128, 2, 512], F32, tag="sm")
                            nc.vector.scalar_tensor_tensor(
                                sm[:], pss[:], rkinv8[:, jc:jc + 1],
                                mkt[:, None, :].to_broadcast((128, 2, 512)),
                                AL.mult, AL.add,
                            )
                            nc.scalar.activation(
                                pT_t[:, 2 * pair:2 * pair + 2, :], sm[:], AF.Exp
                            )
                        else:
                            nc.scalar.activation(
                                pT_t[:, 2 * pair:2 * pair + 2, :], pss[:], AF.Exp,
                                scale=rkinv8[:, jc:jc + 1],
                            )
                    if debug and ic == 0 and jc < 3:
                        nc.sync.dma_start(out=d_pT[jc], in_=pT_t[:])
                    pump(wkv_, 5 if jc < 6 else 4)
                    pump(wpv, 2 if jc < 10 else 4)
                    if ic < 3 and jc == 9:
                        pump(wq_, len(wq_))
                        q_fin(ic + 1, nrms, nqros)
                    else:
                        pump(wq_, 1 if jc < 4 else 3)

            if debug:
                nc.sync.dma_start(out=d_qT[:], in_=qT[:])
                nc.sync.dma_start(out=d_kT[:], in_=kT[:])
                nc.sync.dma_start(out=d_rk[:], in_=rkinv8[:])
                nc.sync.dma_start(out=d_v[:], in_=v_sb[:])

            # tail: last block's PV sweeps + finishes
            pump(wpv, len(wpv))
            pv = state[3]
            for sub in range(4):
                ob = []
                for hd in range(4):
                    pv_one(pv["pts"], sub, hd, ("orA", "orB")[(4 * sub + hd) % 2],
                           ob, False)()
                sub_fin(12 + sub, ob, tail=True)()

    nc.compile()
    return nc


def _get(use_mask: bool, use_bias: bool = False):
    key = (use_mask, use_bias)
    if key not in _cache:
        _cache[key] = _build(use_mask, use_bias)
    return _cache[key]


def _host_prep(hidden_state, attention_mask, Wq, bq, Wk, bk, Wv, bv, Wo,
               use_mask, use_bias):
    """Build the 8 per-core input maps."""
    half_q, half_k = HID // 2, (HKV * D) // 2  # 512, 128
    inv_q = ROPE_BASE ** (-np.arange(half_q, dtype=np.float64) / half_q)
    inv_k = ROPE_BASE ** (-np.arange(half_k, dtype=np.float64) / half_k)
    s_idx = np.arange(S, dtype=np.float64)
    ang_q = inv_q[:, None] * s_idx[None, :]  # [512, S] freq-major
    ang_k = inv_k[:, None] * s_idx[None, :]  # [128, S]
    cos_q, sin_q = np.cos(ang_q), np.sin(ang_q)
    cos_k, sin_k = np.cos(ang_k), np.sin(ang_k)

    in_maps = []
    for core in range(8):
        b, j = core // 4, core % 4
        own_q = np.arange(j * 256, (j + 1) * 256)
        par_q = own_q + 512 if j < 2 else own_q - 512
        fidx_q = own_q if j < 2 else own_q - 512
        sign = -1.0 if j < 2 else 1.0
        own_k = np.arange(j * 64, (j + 1) * 64)
        par_k = own_k + 128 if j < 2 else own_k - 128
        fidx_k = own_k if j < 2 else own_k - 128

        hTc = np.ascontiguousarray(hidden_state[b].T).astype(NB).reshape(8, 128, S)
        wq_c = np.concatenate([Wq[:, own_q], Wq[:, par_q]], axis=1)
        wq_c = wq_c.astype(NB).reshape(8, 128, QSEL)
        wkv_c = np.concatenate(
            [Wk[:, own_k], Wk[:, par_k], Wv[:, own_k]], axis=1
        ).astype(NB).reshape(8, 128, 192)
        wo_c = Wo[j * 256:(j + 1) * 256, :].astype(NB).reshape(2, 128, HID)
        # natural-layout cos/sin: [sc, seq128, {cos, signed sin}, feat]
        csq_c = np.stack(
            [cos_q[fidx_q].T, (sign * sin_q[fidx_q]).T], axis=1
        )  # [S, 2, 256]
        csq_c = csq_c.astype(NB).reshape(NSC, 128, 2, 256)
        csk_c = np.stack(
            [cos_k[fidx_k].T, (sign * sin_k[fidx_k]).T], axis=1
        )  # [S, 2, 64]
        csk_c = csk_c.astype(NB).reshape(NSC, 128, 2, 64)

        m = {
            "hT": hTc, "wq": wq_c, "wkv": wkv_c, "wo": wo_c,
            "csq": csq_c, "csk": csk_c,
        }
        if use_mask:
            mT = np.ascontiguousarray(attention_mask[b].T).astype(np.float32)
            m["mk"] = mT.reshape(NSC, 128, S)
        if use_bias:
            m["brq"] = np.concatenate([bq[own_q], bq[par_q]]).astype(NB).reshape(1, QSEL)
            m["brkv"] = np.concatenate(
                [bk[own_k], bk[par_k], bv[own_k]]
            ).astype(NB).reshape(1, 192)
        in_maps.append(m)
    return in_maps


def kernel(hidden_state, attention_mask, Wq, bq, Wk, bk, Wv, bv, Wo, bo):
    from concourse.bass_utils import run_bass_kernel_spmd

    hidden_state = np.asarray(hidden_state, dtype=np.float32)
    attention_mask = np.asarray(attention_mask, dtype=np.float32)
    Wq, bq = np.asarray(Wq, np.float32), np.asarray(bq, np.float32)
    Wk, bk = np.asarray(Wk, np.float32), np.asarray(bk, np.float32)
    Wv, bv = np.asarray(Wv, np.float32), np.asarray(bv, np.float32)
    Wo, bo = np.asarray(Wo, np.float32), np.asarray(bo, np.float32)
    use_mask = bool(np.any(attention_mask))
    use_bias = bool(np.any(bq) or np.any(bk) or np.any(bv))
    nc = _get(use_mask, use_bias)
    in_maps = _host_prep(
        hidden_state, attention_mask, Wq, bq, Wk, bk, Wv, bv, Wo,
        use_mask, use_bias,
    )
    res = run_bass_kernel_spmd(nc, in_maps, list(range(8)))
    out = np.zeros((B, S, HID), dtype=np.float32)
    for core in range(8):
        out[core // 4] += res.results[core]["y"].astype(np.float32).reshape(S, HID)
    out += bo[None, None, :]
    return out
